# revision 26
# baseline (speedup 1.0000x reference)
"""Bradley-Terry loss kernel for Trainium2 — fp8 DoubleRow Chebyshev design.

loss = sum_{i!=j} W[i,j] * softplus(b_j - b_i)
     = sum_{m,l} A[m,l] * z[m,l] - ln2 * trace(W),
  z[m,l] = sum_ij W_ij T_m(x_i) T_l(x_j),  x = (b - c)/h in [-1,1]

softplus(h*(y-x)) is approximated by a degree-31 tensor-product Chebyshev
expansion (approx error ~6e-8 end-to-end).  Per core, TensorE computes
  Y[m, j] = sum_{i in shard} W[i, j] * T_m(x_i)
with the Chebyshev basis as the stationary operand in fp8(e4m3) DoubleRow
mode (two contraction rows per cycle, 256 W-rows per matmul group).  The
basis is kept at double-fp8 precision by stacking hi/lo columns
[C_hi | C_lo] -> M=64, which fits DoubleRow's 64-partition output limit.
W itself streams as fp8(e4m3): quantization error on U[0,1] entries is
zero-mean and washes out over the 67M-term sum (measured ~1e-4 rel end
to end, vs the 2e-2 gate).

The j-contraction with D[m,j] = sum_l A[m,l] T_l(x_j) (computed on host
in f64, shipped fp8) runs on-device: VectorE multiplies each PSUM slab
by D and row-reduces, so only a tiny [64, 16] accumulator leaves the
chip instead of a 4MB Y, streamed out four columns at a time as each
generation drains (the end-of-kernel DMA waits on 4 drains, not 16).  D is stored 128-partitions-wide (column halves
stacked) so its load uses all 16 DMA engines; drains for the upper
column half read it at base partition 64 (legal: in0 is PSUM).

W is pre-interleaved on the host to [p][ch][t2][r][j] so every W load is
a plain 4KB-per-partition contiguous read on the sync HWDGE queue; D and
the basis ride the Activation queue, keeping the W stream uninterrupted.
Total HBM traffic per core ~9.1MB (vs 36MB for the f32 baseline).
"""

import numpy as np
import ml_dtypes

import concourse.bacc as bacc
import concourse.bass as bass
import concourse.mybir as mybir
from concourse import tile
from concourse.bass_utils import run_bass_kernel_spmd

N = 8192
NCORES = 8
R = N // NCORES            # 1024 rows per core
P = 128                    # SBUF partitions
T2 = R // (2 * P)          # 4 double-row tiles of 256 rows per core
CHALF = 2048               # column group processed per PSUM generation
NHALF = N // CHALF
SLAB = 512                 # PSUM bank free size (fp32)
NSLAB = CHALF // SLAB      # 4 tags x 2 bufs -> 8 PSUM banks
HALFC = N // 2             # 4096: D column-halves stacking offset
DEG = 31
M1 = DEG + 1               # 32 chebyshev coefficients
MSTACK = 2 * M1            # hi|lo stacked -> 64 matmul output partitions
_NEG_LN2 = -float(np.log(2.0))

_cached_nc = None


def _cheb_vals(x, deg):
    out = np.empty((len(x), deg + 1), dtype=np.float64)
    out[:, 0] = 1.0
    if deg >= 1:
        out[:, 1] = x
    for k in range(2, deg + 1):
        out[:, k] = 2 * x * out[:, k - 1] - out[:, k - 2]
    return out


def _cheb2d_coeffs(f, deg):
    n = deg + 1
    theta = (np.arange(n) + 0.5) * np.pi / n
    pts = np.cos(theta)
    F = f(pts[:, None], pts[None, :])
    Tm = np.cos(np.outer(np.arange(n), theta))
    A = (2.0 / n) * Tm @ F @ ((2.0 / n) * Tm).T
    A[0, :] /= 2
    A[:, 0] /= 2
    return A


def _build():
    nc = bacc.Bacc(
        "TRN2",
        target_bir_lowering=False,
        debug=False,
        enable_asserts=False,
        num_devices=NCORES,
    )
    f32 = mybir.dt.float32
    bf16 = mybir.dt.bfloat16
    fp8 = mybir.dt.float8e4
    # w[p, ((ch*T2 + t2)*2 + r)*CHALF + j] = W[t2*256 + r*128 + p, ch*CHALF + j]
    w = nc.dram_tensor("w", [P, NHALF * T2 * 2 * CHALF], fp8, kind="ExternalInput")
    crows = nc.dram_tensor("crows", [P, T2, 2, MSTACK], fp8, kind="ExternalInput")
    # dmat[p, j] = D'[p, j] for p<64 else D'[p-64, HALFC+j];  D' = [D31; D31]
    dmat = nc.dram_tensor("dmat", [P, HALFC], fp8, kind="ExternalInput")
    diag = nc.dram_tensor("diag", [P, R // P], f32, kind="ExternalInput")
    acc_out = nc.dram_tensor("acc", [MSTACK, NHALF * NSLAB], f32, kind="ExternalOutput")
    dsum = nc.dram_tensor("dsum", [P, 1], f32, kind="ExternalOutput")

    with tile.TileContext(nc) as tc:
        with (
            tc.tile_pool(name="consts", bufs=1) as consts,
            tc.tile_pool(name="wpool", bufs=8) as wpool,
            tc.tile_pool(name="wsplit", bufs=4) as wsplit,
            tc.tile_pool(name="psum", bufs=4, space="PSUM") as pspool,
            tc.tile_pool(name="scr", bufs=2) as scrpool,
            tc.tile_pool(name="small", bufs=2) as small,
        ):
            crows_sb = consts.tile([P, T2, 2, MSTACK], fp8)
            nc.scalar.dma_start(crows_sb[:], crows.ap())
            diag_sb = consts.tile([P, R // P], f32)
            nc.gpsimd.dma_start(diag_sb[:], diag.ap())
            # D rides the Activation HWDGE queue so the sync queue stays a
            # pure W-read stream.
            dmat_sb = consts.tile([P, HALFC], fp8)
            nc.scalar.dma_start(dmat_sb[:], dmat.ap())
            acc = consts.tile([MSTACK, NHALF * NSLAB], f32)

            # ~3.4us of throwaway matmuls on the already-resident basis
            # during the DMA ramp: flips the PE_HAM clock gate to 8/8
            # (2.4 GHz) before the first real matmul, so the W-paced main
            # loop never runs at the 1.2 GHz cold clock.  Output goes to
            # the ps0-tag PSUM bank; the real generation 0 clears it via
            # start=True.
            warm_ps = pspool.tile([MSTACK, 256], f32, tag="ps0", name="warm")
            warm_rhs = crows_sb.rearrange("p t r m -> p r t m")
            for _ in range(16):
                nc.tensor.matmul(
                    warm_ps[:],
                    crows_sb[:, 0, :, :],
                    warm_rhs,
                    start=True,
                    stop=True,
                    perf_mode=mybir.MatmulPerfMode.DoubleRow,
                )

            # Generations are 1024 columns (2 slabs) with a 4-deep PSUM
            # rotation per tag (2 tags x 4 bufs = 8 banks): generation
            # g+4 -- not g+2 -- waits on g's drains, so the drain chain
            # never gates the matmul stream or the tail.
            def drain(ch, h, s, ps_tile):
                scr = scrpool.tile([MSTACK, SLAB], f32, tag=f"scr{h}")
                col = ch * NSLAB + h * 2 + s
                c0 = ch * CHALF + h * 1024 + s * SLAB
                if c0 < HALFC:
                    d_in1 = dmat_sb[:MSTACK, c0 : c0 + SLAB]
                else:
                    d_in1 = dmat_sb[MSTACK:, c0 - HALFC : c0 - HALFC + SLAB]
                nc.vector.scalar_tensor_tensor(
                    out=scr[:],
                    in0=ps_tile[:],
                    scalar=1.0,
                    in1=d_in1,
                    op0=mybir.AluOpType.mult,
                    op1=mybir.AluOpType.mult,
                    accum_out=acc[:, col : col + 1],
                )

            for ch in range(NHALF - 1):
                ps = {}
                for h in range(2):
                    for s in range(2):
                        ps[(h, s)] = pspool.tile(
                            [MSTACK, SLAB], f32, tag=f"ps{s}", name=f"ps{s}_{ch}_{h}"
                        )
                for t2 in range(T2):
                    wt = wpool.tile([P, 2, CHALF], fp8, tag="w")
                    off = (ch * T2 + t2) * 2 * CHALF
                    nc.sync.dma_start(
                        wt[:],
                        w.ap()[:, off : off + 2 * CHALF].rearrange(
                            "p (r j) -> p r j", r=2
                        ),
                    )
                    lhsT = crows_sb[:, t2, :, :]
                    for h in range(2):
                        for s in range(2):
                            c = h * 1024 + s * SLAB
                            nc.tensor.matmul(
                                ps[(h, s)][:],
                                lhsT,
                                wt[:, :, c : c + SLAB],
                                start=(t2 == 0),
                                stop=(t2 == T2 - 1),
                                perf_mode=mybir.MatmulPerfMode.DoubleRow,
                            )
                for h in range(2):
                    for s in range(2):
                        drain(ch, h, s, ps[(h, s)])
                # stream this generation's accumulator columns out now so
                # the end-of-kernel DMA only waits on the last two
                nc.scalar.dma_start(
                    acc_out.ap()[:, ch * NSLAB : (ch + 1) * NSLAB],
                    acc[:, ch * NSLAB : (ch + 1) * NSLAB],
                )

            # Last column group runs as two half-width passes so the
            # post-stream tail is only 2 matmuls + 2 drains + a 2-column
            # accumulator DMA; the first pass's drains overlap the second
            # pass's W stream.
            ch = NHALF - 1
            for h in range(2):
                ps = {
                    s: pspool.tile(
                        [MSTACK, SLAB], f32, tag=f"ps{s}", name=f"ps{s}_{ch}_{h}"
                    )
                    for s in range(2)
                }
                for t2 in range(T2):
                    wt = wsplit.tile([P, 2, 1024], fp8, tag="ws")
                    off = (ch * T2 + t2) * 2 * CHALF
                    nc.sync.dma_start(
                        wt[:],
                        w.ap()[:, off : off + 2 * CHALF].rearrange(
                            "p (r j) -> p r j", r=2
                        )[:, :, h * 1024 : (h + 1) * 1024],
                    )
                    lhsT = crows_sb[:, t2, :, :]
                    for s in range(2):
                        nc.tensor.matmul(
                            ps[s][:],
                            lhsT,
                            wt[:, :, s * SLAB : (s + 1) * SLAB],
                            start=(t2 == 0),
                            stop=(t2 == T2 - 1),
                            perf_mode=mybir.MatmulPerfMode.DoubleRow,
                        )
                for s in range(2):
                    drain(ch, h, s, ps[s])
                nc.scalar.dma_start(
                    acc_out.ap()[:, ch * NSLAB + h * 2 : ch * NSLAB + h * 2 + 2],
                    acc[:, ch * NSLAB + h * 2 : ch * NSLAB + h * 2 + 2],
                )

            # dsum[p] = -ln2 * sum_t diag[p, t]
            dscr = small.tile([P, R // P], f32, tag="dscr")
            dacc = small.tile([P, 1], f32, tag="dacc")
            nc.vector.scalar_tensor_tensor(
                out=dscr[:],
                in0=diag_sb[:],
                scalar=_NEG_LN2,
                in1=diag_sb[:],
                op0=mybir.AluOpType.mult,
                op1=mybir.AluOpType.bypass,
                accum_out=dacc[:],
            )
            nc.sync.dma_start(dsum.ap(), dacc[:])

    nc.compile()
    return nc


def _get_nc():
    global _cached_nc
    if _cached_nc is None:
        _cached_nc = _build()
    return _cached_nc


def kernel(win_matrix, betas, _trace=False):
    win_matrix = np.asarray(win_matrix, dtype=np.float32)
    betas = np.asarray(betas, dtype=np.float32)
    nc = _get_nc()

    b64 = betas.astype(np.float64)
    lo, hi = float(b64.min()), float(b64.max())
    c = 0.5 * (lo + hi)
    h = max(0.5 * (hi - lo) * 1.000001, 1e-12)
    x = (b64 - c) / h
    A = _cheb2d_coeffs(lambda X, Y: np.logaddexp(0.0, h * (Y - X)), DEG)
    C = _cheb_vals(x, DEG)                       # [N, 32] f64
    fp8 = ml_dtypes.float8_e4m3
    C_hi = C.astype(fp8)
    C_lo = (C - C_hi.astype(np.float64)).astype(fp8)
    C_st = np.concatenate([C_hi, C_lo], axis=1)  # [N, 64] fp8
    D31 = A @ C.T                                # [32, N] f64
    Dp = np.concatenate([D31, D31], axis=0)      # [64, N] f64
    Dpad = np.ascontiguousarray(
        np.concatenate([Dp[:, :HALFC], Dp[:, HALFC:]], axis=0).astype(fp8)
    )                                            # [128, 4096] fp8

    W8 = win_matrix.astype(fp8)                  # [N, N] fp8
    dvals = np.ascontiguousarray(np.diagonal(win_matrix))
    in_maps = []
    for cc in range(NCORES):
        rows = slice(cc * R, (cc + 1) * R)
        # w_host[p, ch, t2, r, j] = W8[cc*R + t2*256 + r*128 + p, ch*CHALF + j]
        w_np = np.ascontiguousarray(
            W8[rows]
            .reshape(T2, 2, P, NHALF, CHALF)
            .transpose(2, 3, 0, 1, 4)
            .reshape(P, NHALF * T2 * 2 * CHALF)
        )
        # crows[p, t2, r, m] = C_st[cc*R + t2*256 + r*128 + p, m]
        crows_np = np.ascontiguousarray(
            C_st[rows].reshape(T2, 2, P, MSTACK).transpose(2, 0, 1, 3)
        )
        in_maps.append(
            {
                "w": w_np,
                "crows": crows_np,
                "dmat": Dpad,
                "diag": np.ascontiguousarray(
                    dvals[rows].reshape(R // P, P).T.astype(np.float32)
                ),
            }
        )
    res = run_bass_kernel_spmd(
        nc, in_maps, core_ids=list(range(NCORES)), trace=_trace
    )

    total = 0.0
    for cc in range(NCORES):
        total += float(res.results[cc]["acc"].astype(np.float64).sum())
        total += float(res.results[cc]["dsum"].astype(np.float64).sum())
    if _trace:
        kernel.last_results = res
    return np.array(total, dtype=np.float32)


# revision 29
# speedup vs baseline: 1.0525x; 1.0525x over previous
"""Bradley-Terry loss kernel for Trainium2 — fp8 DoubleRow Chebyshev design.

loss = sum_{i!=j} W[i,j] * softplus(b_j - b_i)
     = sum_{m,l} A[m,l] * z[m,l] - ln2 * trace(W),
  z[m,l] = sum_ij W_ij T_m(x_i) T_l(x_j),  x = (b - c)/h in [-1,1]

softplus(h*(y-x)) is approximated by a degree-31 tensor-product Chebyshev
expansion (approx error ~6e-8 end-to-end).  Per core, TensorE computes
  Y[m, j] = sum_{i in shard} W[i, j] * T_m(x_i)
with the Chebyshev basis as the stationary operand in fp8(e4m3) DoubleRow
mode (two contraction rows per cycle, 256 W-rows per matmul group).  The
basis is kept at double-fp8 precision by stacking hi/lo columns
[C_hi | C_lo] -> M=64, which fits DoubleRow's 64-partition output limit.
W itself streams as fp8(e4m3): quantization error on U[0,1] entries is
zero-mean and washes out over the 67M-term sum (measured ~1e-4 rel end
to end, vs the 2e-2 gate).

The j-contraction with D[m,j] = sum_l A[m,l] T_l(x_j) (computed on host
in f64, shipped fp8) runs on-device: VectorE multiplies each PSUM slab
by D and row-reduces, so only a tiny [64, 16] accumulator leaves the
chip instead of a 4MB Y, streamed out four columns at a time as each
generation drains (the end-of-kernel DMA waits on 4 drains, not 16).  D is stored 128-partitions-wide (column halves
stacked) so its load uses all 16 DMA engines; drains for the upper
column half read it at base partition 64 (legal: in0 is PSUM).

W is pre-interleaved on the host to [p][ch][t2][r][j] so every W load is
a plain 4KB-per-partition contiguous read on the sync HWDGE queue; D and
the basis ride the Activation queue, keeping the W stream uninterrupted.
Total HBM traffic per core ~9.1MB (vs 36MB for the f32 baseline).
"""

import numpy as np
import ml_dtypes

import concourse.bacc as bacc
import concourse.bass as bass
import concourse.mybir as mybir
from concourse import tile
from concourse.bass_utils import run_bass_kernel_spmd

N = 8192
NCORES = 8
R = N // NCORES            # 1024 rows per core
P = 128                    # SBUF partitions
T2 = R // (2 * P)          # 4 double-row tiles of 256 rows per core
CHALF = 2048               # column group processed per PSUM generation
NHALF = N // CHALF
SLAB = 512                 # PSUM bank free size (fp32)
NSLAB = CHALF // SLAB      # 4 tags x 2 bufs -> 8 PSUM banks
HALFC = N // 2             # 4096: D column-halves stacking offset
DEG = 31
M1 = DEG + 1               # 32 chebyshev coefficients
MSTACK = 2 * M1            # hi|lo stacked -> 64 matmul output partitions
_NEG_LN2 = -float(np.log(2.0))

_cached_nc = None


def _cheb_vals(x, deg):
    out = np.empty((len(x), deg + 1), dtype=np.float64)
    out[:, 0] = 1.0
    if deg >= 1:
        out[:, 1] = x
    for k in range(2, deg + 1):
        out[:, k] = 2 * x * out[:, k - 1] - out[:, k - 2]
    return out


def _cheb2d_coeffs(f, deg):
    n = deg + 1
    theta = (np.arange(n) + 0.5) * np.pi / n
    pts = np.cos(theta)
    F = f(pts[:, None], pts[None, :])
    Tm = np.cos(np.outer(np.arange(n), theta))
    A = (2.0 / n) * Tm @ F @ ((2.0 / n) * Tm).T
    A[0, :] /= 2
    A[:, 0] /= 2
    return A


def _build():
    nc = bacc.Bacc(
        "TRN2",
        target_bir_lowering=False,
        debug=False,
        enable_asserts=False,
        num_devices=NCORES,
    )
    f32 = mybir.dt.float32
    bf16 = mybir.dt.bfloat16
    fp8 = mybir.dt.float8e4
    # w[p, ((ch*T2 + t2)*2 + r)*CHALF + j] = W[t2*256 + r*128 + p, ch*CHALF + j]
    w = nc.dram_tensor("w", [P, NHALF * T2 * 2 * CHALF], fp8, kind="ExternalInput")
    crows = nc.dram_tensor("crows", [P, T2, 2, MSTACK], fp8, kind="ExternalInput")
    # dmat[p, j] = D'[p, j] for p<64 else D'[p-64, HALFC+j];  D' = [D31; D31]
    dmat = nc.dram_tensor("dmat", [P, HALFC], fp8, kind="ExternalInput")
    diag = nc.dram_tensor("diag", [P, R // P], f32, kind="ExternalInput")
    acc_out = nc.dram_tensor("acc", [MSTACK, NHALF * NSLAB], f32, kind="ExternalOutput")
    dsum = nc.dram_tensor("dsum", [P, 1], f32, kind="ExternalOutput")

    with tile.TileContext(nc) as tc:
        with (
            tc.tile_pool(name="consts", bufs=1) as consts,
            tc.tile_pool(name="wpool", bufs=8) as wpool,
            tc.tile_pool(name="psum", bufs=4, space="PSUM") as pspool,
            tc.tile_pool(name="scr", bufs=2) as scrpool,
            tc.tile_pool(name="small", bufs=2) as small,
        ):
            crows_sb = consts.tile([P, T2, 2, MSTACK], fp8)
            nc.scalar.dma_start(crows_sb[:], crows.ap())
            diag_sb = consts.tile([P, R // P], f32)
            nc.gpsimd.dma_start(diag_sb[:], diag.ap())
            # D rides the Activation HWDGE queue so the sync queue stays a
            # pure W-read stream.
            dmat_sb = consts.tile([P, HALFC], fp8)
            nc.scalar.dma_start(dmat_sb[:], dmat.ap())
            acc = consts.tile([MSTACK, NHALF * NSLAB], f32)

            # ~3.4us of throwaway matmuls on the already-resident basis
            # during the DMA ramp: flips the PE_HAM clock gate to 8/8
            # (2.4 GHz) before the first real matmul, so the W-paced main
            # loop never runs at the 1.2 GHz cold clock.  Output goes to
            # the ps0-tag PSUM bank; the real generation 0 clears it via
            # start=True.
            warm_ps = pspool.tile([MSTACK, 256], f32, tag="ps0", name="warm")
            warm_rhs = crows_sb.rearrange("p t r m -> p r t m")
            for _ in range(16):
                nc.tensor.matmul(
                    warm_ps[:],
                    crows_sb[:, 0, :, :],
                    warm_rhs,
                    start=True,
                    stop=True,
                    perf_mode=mybir.MatmulPerfMode.DoubleRow,
                )

            # Generations are 1024 columns (2 slabs) with a 4-deep PSUM
            # rotation per tag (2 tags x 4 bufs = 8 banks): generation
            # g+4 -- not g+2 -- waits on g's drains, so the drain chain
            # never gates the matmul stream or the tail.
            def drain(ch, h, s, ps_tile):
                scr = scrpool.tile([MSTACK, SLAB], f32, tag=f"scr{h}")
                col = ch * NSLAB + h * 2 + s
                c0 = ch * CHALF + h * 1024 + s * SLAB
                if c0 < HALFC:
                    d_in1 = dmat_sb[:MSTACK, c0 : c0 + SLAB]
                else:
                    d_in1 = dmat_sb[MSTACK:, c0 - HALFC : c0 - HALFC + SLAB]
                nc.vector.scalar_tensor_tensor(
                    out=scr[:],
                    in0=ps_tile[:],
                    scalar=1.0,
                    in1=d_in1,
                    op0=mybir.AluOpType.mult,
                    op1=mybir.AluOpType.mult,
                    accum_out=acc[:, col : col + 1],
                )

            for ch in range(NHALF):
                ps = {}
                for h in range(2):
                    for s in range(2):
                        ps[(h, s)] = pspool.tile(
                            [MSTACK, SLAB], f32, tag=f"ps{s}", name=f"ps{s}_{ch}_{h}"
                        )
                for t2 in range(T2):
                    wt = wpool.tile([P, 2, CHALF], fp8, tag="w")
                    off = (ch * T2 + t2) * 2 * CHALF
                    nc.sync.dma_start(
                        wt[:],
                        w.ap()[:, off : off + 2 * CHALF].rearrange(
                            "p (r j) -> p r j", r=2
                        ),
                    )
                    lhsT = crows_sb[:, t2, :, :]
                    for h in range(2):
                        for s in range(2):
                            c = h * 1024 + s * SLAB
                            nc.tensor.matmul(
                                ps[(h, s)][:],
                                lhsT,
                                wt[:, :, c : c + SLAB],
                                start=(t2 == 0),
                                stop=(t2 == T2 - 1),
                                perf_mode=mybir.MatmulPerfMode.DoubleRow,
                            )
                for h in range(2):
                    for s in range(2):
                        drain(ch, h, s, ps[(h, s)])
                # stream this generation's accumulator columns out now so
                # the end-of-kernel DMA only waits on the last four
                nc.scalar.dma_start(
                    acc_out.ap()[:, ch * NSLAB : (ch + 1) * NSLAB],
                    acc[:, ch * NSLAB : (ch + 1) * NSLAB],
                )

            # dsum[p] = -ln2 * sum_t diag[p, t]
            dscr = small.tile([P, R // P], f32, tag="dscr")
            dacc = small.tile([P, 1], f32, tag="dacc")
            nc.vector.scalar_tensor_tensor(
                out=dscr[:],
                in0=diag_sb[:],
                scalar=_NEG_LN2,
                in1=diag_sb[:],
                op0=mybir.AluOpType.mult,
                op1=mybir.AluOpType.bypass,
                accum_out=dacc[:],
            )
            nc.sync.dma_start(dsum.ap(), dacc[:])

    nc.compile()
    return nc


def _get_nc():
    global _cached_nc
    if _cached_nc is None:
        _cached_nc = _build()
    return _cached_nc


def kernel(win_matrix, betas, _trace=False):
    win_matrix = np.asarray(win_matrix, dtype=np.float32)
    betas = np.asarray(betas, dtype=np.float32)
    nc = _get_nc()

    b64 = betas.astype(np.float64)
    lo, hi = float(b64.min()), float(b64.max())
    c = 0.5 * (lo + hi)
    h = max(0.5 * (hi - lo) * 1.000001, 1e-12)
    x = (b64 - c) / h
    A = _cheb2d_coeffs(lambda X, Y: np.logaddexp(0.0, h * (Y - X)), DEG)
    C = _cheb_vals(x, DEG)                       # [N, 32] f64
    fp8 = ml_dtypes.float8_e4m3
    C_hi = C.astype(fp8)
    C_lo = (C - C_hi.astype(np.float64)).astype(fp8)
    C_st = np.concatenate([C_hi, C_lo], axis=1)  # [N, 64] fp8
    D31 = A @ C.T                                # [32, N] f64
    Dp = np.concatenate([D31, D31], axis=0)      # [64, N] f64
    Dpad = np.ascontiguousarray(
        np.concatenate([Dp[:, :HALFC], Dp[:, HALFC:]], axis=0).astype(fp8)
    )                                            # [128, 4096] fp8

    W8 = win_matrix.astype(fp8)                  # [N, N] fp8
    dvals = np.ascontiguousarray(np.diagonal(win_matrix))
    in_maps = []
    for cc in range(NCORES):
        rows = slice(cc * R, (cc + 1) * R)
        # w_host[p, ch, t2, r, j] = W8[cc*R + t2*256 + r*128 + p, ch*CHALF + j]
        w_np = np.ascontiguousarray(
            W8[rows]
            .reshape(T2, 2, P, NHALF, CHALF)
            .transpose(2, 3, 0, 1, 4)
            .reshape(P, NHALF * T2 * 2 * CHALF)
        )
        # crows[p, t2, r, m] = C_st[cc*R + t2*256 + r*128 + p, m]
        crows_np = np.ascontiguousarray(
            C_st[rows].reshape(T2, 2, P, MSTACK).transpose(2, 0, 1, 3)
        )
        in_maps.append(
            {
                "w": w_np,
                "crows": crows_np,
                "dmat": Dpad,
                "diag": np.ascontiguousarray(
                    dvals[rows].reshape(R // P, P).T.astype(np.float32)
                ),
            }
        )
    res = run_bass_kernel_spmd(
        nc, in_maps, core_ids=list(range(NCORES)), trace=_trace
    )

    total = 0.0
    for cc in range(NCORES):
        total += float(res.results[cc]["acc"].astype(np.float64).sum())
        total += float(res.results[cc]["dsum"].astype(np.float64).sum())
    if _trace:
        kernel.last_results = res
    return np.array(total, dtype=np.float32)
